# Initial kernel scaffold
#
"""Trainium2 Bass kernel for nn_DiagnoerMinBlcokScan (grouped 1D conv,
G=8 groups x FG=16 filters x J=8 channels, W=31 window, L=262144).

Strategy: data-parallel over L across 8 cores (no collectives; host slices
haloed shards). Inside each core the conv is phase-packed: output phases
o=0..7 fold into the matmul M dim (M = 16 filters x 8 phases = 128) and the
input is 8-phase deinterleaved so each of 5 "q" matmuls (contract = 64
(j,r) rows) reads a plain contiguous slice of SBUF. PSUM accumulates the 5
matmuls; result [128, n] tiles stream back to HBM. Group pairs share the
128-partition SBUF tiles (lower/upper 64 rows) so DMA runs full-width and
the PE can overlap row-disjoint matmuls.

Self-contained: hardcodes all shapes; host does the cheap boundary columns
(truncated-window semantics of the reference) and the phase re-interleave.
"""
import numpy as np

import concourse.bass as bass
import concourse.mybir as mybir
from concourse.bass_utils import run_bass_kernel_spmd
from concourse.tile import TileContext

D, L = 64, 262144
G, J, FG, W = 8, 8, 16, 31
PAD1 = 15
F = G * FG
NCORES = 8
LS = L // NCORES            # 32768 output cols per core
M = LS // 8                 # 4096 matmul free positions per core
MH = M + 5                  # deinterleaved cols incl. halo (n+q, q in 0..4)
NT = 512                    # matmul free-dim tile (one PSUM bank)
NTILES = M // NT            # 8

MM_DT = mybir.dt.float32r   # full-rate fp32 path on the PE array
F32 = mybir.dt.float32

_cache = {}


def _build_bass():
    nc = bass.Bass()
    xb_h = nc.declare_dram_parameter("xb", [4, 128, MH], MM_DT, isOutput=False)
    w_h = nc.declare_dram_parameter("w", [4, 5, 128, 128], MM_DT, isOutput=False)
    y_h = nc.declare_dram_parameter("y", [G, 128, M], F32, isOutput=True)

    with TileContext(nc) as tc:
        with (
            tc.tile_pool(name="wpool", bufs=1) as wp,
            tc.tile_pool(name="xpool", bufs=2) as xp,
            tc.tile_pool(name="psum", bufs=4, space="PSUM") as pp,
            tc.tile_pool(name="ypool", bufs=4) as yp,
        ):
            # all weights resident: [128, 20*128] fp32 = 1.3 MB
            wt = wp.tile([128, 4 * 5 * 128], MM_DT)
            nc.sync.dma_start(out=wt[:], in_=w_h.rearrange("t q p m -> p (t q m)"))

            ncopy = 0
            for t in range(4):
                xt = xp.tile([128, MH], MM_DT)
                nc.sync.dma_start(out=xt[:], in_=xb_h[t])
                for i in range(NTILES):
                    n0 = NT * i
                    for half in range(2):
                        g = 2 * t + half
                        sl = slice(64 * half, 64 * half + 64)
                        ps = pp.tile([128, NT], F32)
                        for q in range(5):
                            nc.tensor.matmul(
                                ps[:],
                                wt[sl, (t * 5 + q) * 128:(t * 5 + q) * 128 + 128],
                                xt[sl, n0 + q: n0 + q + NT],
                                start=(q == 0),
                                stop=(q == 4),
                            )
                        yt = yp.tile([128, NT], F32)
                        eng = nc.vector if ncopy % 2 == 0 else nc.scalar
                        eng.tensor_copy(out=yt[:], in_=ps[:])
                        ncopy += 1
                        nc.sync.dma_start(out=y_h[g, :, n0:n0 + NT], in_=yt[:])
    return nc


def _host_prep(x, params, rel_idx):
    x2 = np.ascontiguousarray(np.asarray(x, dtype=np.float32)[np.asarray(rel_idx).reshape(-1)])
    x_pad = np.pad(x2, ((0, 0), (PAD1, 17)))                     # [64, L+32]
    xb_cores = []
    for c in range(NCORES):
        xs = x_pad[:, c * LS: c * LS + LS + 32]                  # [64, 8*MH]
        arr = xs.reshape(G, J, MH, 8).transpose(0, 1, 3, 2)      # [g, j, r, m]
        xb_cores.append(np.ascontiguousarray(arr.reshape(4, 128, MH), dtype=np.float32))

    p = np.asarray(params, dtype=np.float32)
    wq = np.zeros((4, 5, 128, 128), dtype=np.float32)
    o_i = np.arange(8)
    r_i = np.arange(8)
    for g in range(G):
        t, half = g // 2, g % 2
        for q in range(5):
            w_mat = 8 * q + r_i[:, None] - o_i[None, :]          # [r, o]
            valid = (w_mat >= 0) & (w_mat <= 30)
            wm = np.where(valid, w_mat, 0)
            blk = p[g][:, :, wm] * valid[None, None]             # [f, j, r, o]
            blk = blk.transpose(1, 2, 0, 3).reshape(64, 128)     # [(j,r), (f,o)]
            wq[t, q, 64 * half:64 * half + 64, :] = blk
    return xb_cores, wq, x2, p


def _host_post(y_cores, x2, p):
    parts = [
        y.reshape(G, FG, 8, M).transpose(0, 1, 3, 2).reshape(G, FG, LS)
        for y in y_cores
    ]
    y_full = np.concatenate(parts, axis=2)                       # [G, FG, L]

    xg = x2.reshape(G, J, L)
    pl = np.einsum("gjw,gfjw->gfw", xg[:, :, :W], p)
    left_c = np.cumsum(pl, axis=-1)
    y_full[:, :, :PAD1] = left_c[:, :, W - PAD1 - 1: W - 1]
    pr = np.einsum("gjw,gfjw->gfw", xg[:, :, L - W:], p)
    right_c = np.cumsum(pr[:, :, ::-1], axis=-1)[:, :, ::-1]
    n_right = W - 1 - PAD1
    y_full[:, :, L - n_right:] = right_c[:, :, 1: W - PAD1]
    return np.ascontiguousarray(y_full.reshape(F * L, 1), dtype=np.float32)


def kernel(x, params, rel_idx, _trace=False, _trace_out=None):
    if "nc" not in _cache:
        _cache["nc"] = _build_bass()
    nc = _cache["nc"]

    xb_cores, wq, x2, p = _host_prep(x, params, rel_idx)
    in_maps = [{"xb": xb_cores[c], "w": wq} for c in range(NCORES)]
    res = run_bass_kernel_spmd(
        nc, in_maps, list(range(NCORES)),
        trace=_trace, **({"tmpdir": _trace_out} if _trace_out else {}),
    )
    if _trace_out is not None:
        _cache["last"] = res
    y_cores = [np.asarray(res.results[c]["y"]) for c in range(NCORES)]
    return _host_post(y_cores, x2, p)


# revision 30
# speedup vs baseline: 24.0309x; 24.0309x over previous
"""Trainium2 Bass kernel for nn_DiagnoerMinBlcokScan (grouped 1D conv,
G=8 groups x FG=16 filters x J=8 channels, W=31 window, L=262144).

Strategy: data-parallel over L across 8 cores (no collectives; host slices
haloed shards). Inside each core the conv is phase-packed: output phases
o=0..7 fold into the matmul M dim (M = 16 filters x 8 phases = 128) and the
input is 8-phase deinterleaved so each of 5 "q" matmuls (contract = 64
(j,r) rows) reads a plain contiguous slice of SBUF. PSUM accumulates the 5
matmuls; result [128, n] tiles stream back to HBM. Group pairs share the
128-partition SBUF tiles (lower/upper 64 rows) so DMA runs full-width and
the PE can overlap row-disjoint matmuls.

Self-contained: hardcodes all shapes; host does the cheap boundary columns
(truncated-window semantics of the reference) and the phase re-interleave.
"""
import numpy as np

import concourse.bacc as bacc
import concourse.bass as bass
import concourse.mybir as mybir
from concourse.bass_utils import run_bass_kernel_spmd
from concourse.tile import TileContext
from concourse.tile_rust import add_dep_helper

D, L = 64, 262144
G, J, FG, W = 8, 8, 16, 31
PAD1 = 15
F = G * FG
NCORES = 8
LS = L // NCORES            # 32768 output cols per core
M = LS // 8                 # 4096 matmul free positions per core
MH = M + 4                  # deinterleaved cols incl. halo (n+q, q in 0..4)
NT = 512                    # matmul free-dim tile (one PSUM bank)
NTILES = M // NT            # 8

import os
_DT_NAME = os.environ.get("KERNEL_MM_DT", "bfloat16")
MM_DT = getattr(mybir.dt, _DT_NAME)
F32 = mybir.dt.float32
_NP_IN_DT = mybir.dt.np(MM_DT)

_cache = {}


def _build_bass(loop_n=None):
    """loop_n: if set, wrap the body in a HW loop (for slope timing)."""
    nc = bacc.Bacc()
    xb_h = nc.declare_dram_parameter("xb", [4, 128, MH], MM_DT, isOutput=False)
    w_h = nc.declare_dram_parameter("w", [128, 4 * 5 * 128], MM_DT, isOutput=False)
    y_h = nc.declare_dram_parameter("y", [G, 128, M], F32, isOutput=True)

    with TileContext(nc) as tc:
        with (
            tc.tile_pool(name="wpool", bufs=1) as wp,
            tc.tile_pool(name="xpool", bufs=2) as xp,
            tc.tile_pool(name="psum", bufs=5, space="PSUM") as pp,
            tc.tile_pool(name="psumb", bufs=1, space="PSUM") as pb,
            tc.tile_pool(name="psumd", bufs=1, space="PSUM") as pd,
            tc.tile_pool(name="ypool", bufs=64) as yp,
        ):
            # all weights resident: [128, 20*128] fp32 = 1.3 MB
            wt = wp.tile([128, 4 * 5 * 128], MM_DT)
            nc.sync.dma_start(out=wt[:], in_=w_h[:])

            # Engine (TPB) instructions may carry at most ONE sync wait, so
            # the structure below keeps every matmul/copy at <=1:
            #  - a throwaway matmul reading only `wt` absorbs the weight-DMA
            #    wait into the PE's vector clock (data-dep waits propagate;
            #    explicit-dep nops do not),
            #  - each pair's first group uses a dedicated PSUM slot whose
            #    release is a pair-old DVE tick the PE has already observed,
            #    so its matmul only waits on the xt DMA,
            #  - y staging tiles are never reused (bufs=64) so copies only
            #    wait on the PE.
            dummy = pd.tile([2, 2], F32)
            dmm = nc.tensor.matmul(dummy[:], wt[0:2, 0:2], wt[0:2, 0:2],
                                   start=True, stop=True)

            import contextlib
            loop_cm = tc.For_i(0, loop_n, 1) if loop_n else contextlib.nullcontext()
            with loop_cm:
                _emit_body(nc, tc, wp, xp, pp, pb, pd, yp, wt, dmm, xb_h, y_h)
    nc.compile()
    return nc


def _emit_body(nc, tc, wp, xp, pp, pb, pd, yp, wt, dmm, xb_h, y_h):
    if True:
        if True:
            ncopy = 0
            for t in range(4):
                xt = xp.tile([128, MH], MM_DT)
                nc.sync.dma_start(out=xt[:], in_=xb_h[t])
                for i in range(NTILES):
                    n0 = NT * i
                    for half in range(2):
                        g = 2 * t + half
                        sl = slice(64 * half, 64 * half + 64)
                        boundary = (i == 0 and half == 0)
                        ps = (pb if boundary else pp).tile([128, NT], F32)
                        for q in range(5):
                            mm = nc.tensor.matmul(
                                ps[:],
                                wt[sl, (t * 5 + q) * 128:(t * 5 + q) * 128 + 128],
                                xt[sl, n0 + q: n0 + q + NT],
                                start=(q == 0),
                                stop=(q == 4),
                            )
                            if t == 0 and boundary and q == 0:
                                add_dep_helper(mm.ins, dmm.ins, sync=False,
                                               reason="order after wt gate")
                        yt = yp.tile([128, NT], F32)
                        if boundary or ncopy % 2 == 0:
                            nc.vector.tensor_copy(out=yt[:], in_=ps[:])
                        else:
                            nc.scalar.copy(out=yt[:], in_=ps[:])
                        ncopy += 1
                        nc.sync.dma_start(out=y_h[g, :, n0:n0 + NT], in_=yt[:])


def _host_prep(x, params, rel_idx):
    x2 = np.ascontiguousarray(np.asarray(x, dtype=np.float32)[np.asarray(rel_idx).reshape(-1)])
    x_pad = np.pad(x2, ((0, 0), (PAD1, 17)))                     # [64, L+32]
    xb_cores = []
    for c in range(NCORES):
        xs = x_pad[:, c * LS: c * LS + LS + 32]                  # [64, 8*MH]
        arr = xs.reshape(G, J, MH, 8).transpose(0, 1, 3, 2)      # [g, j, r, m]
        xb_cores.append(np.ascontiguousarray(
            arr.reshape(4, 128, MH)).astype(_NP_IN_DT))

    p = np.asarray(params, dtype=np.float32)
    wq = np.zeros((4, 5, 128, 128), dtype=np.float32)
    o_i = np.arange(8)
    r_i = np.arange(8)
    for g in range(G):
        t, half = g // 2, g % 2
        for q in range(5):
            w_mat = 8 * q + r_i[:, None] - o_i[None, :]          # [r, o]
            valid = (w_mat >= 0) & (w_mat <= 30)
            wm = np.where(valid, w_mat, 0)
            blk = p[g][:, :, wm] * valid[None, None]             # [f, j, r, o]
            blk = blk.transpose(1, 2, 0, 3).reshape(64, 128)     # [(j,r), (f,o)]
            wq[t, q, 64 * half:64 * half + 64, :] = blk
    # device layout: [128 partitions, (t, q, m) flattened]
    wq = np.ascontiguousarray(
        wq.transpose(2, 0, 1, 3).reshape(128, 4 * 5 * 128)).astype(_NP_IN_DT)
    return xb_cores, wq, x2, p


def _host_post(y_cores, x2, p):
    parts = [
        y.reshape(G, FG, 8, M).transpose(0, 1, 3, 2).reshape(G, FG, LS)
        for y in y_cores
    ]
    y_full = np.concatenate(parts, axis=2)                       # [G, FG, L]

    xg = x2.reshape(G, J, L)
    pl = np.einsum("gjw,gfjw->gfw", xg[:, :, :W], p)
    left_c = np.cumsum(pl, axis=-1)
    y_full[:, :, :PAD1] = left_c[:, :, W - PAD1 - 1: W - 1]
    pr = np.einsum("gjw,gfjw->gfw", xg[:, :, L - W:], p)
    right_c = np.cumsum(pr[:, :, ::-1], axis=-1)[:, :, ::-1]
    n_right = W - 1 - PAD1
    y_full[:, :, L - n_right:] = right_c[:, :, 1: W - PAD1]
    return np.ascontiguousarray(y_full.reshape(F * L, 1), dtype=np.float32)


def kernel(x, params, rel_idx, _trace=False, _trace_out=None):
    if "nc" not in _cache:
        _cache["nc"] = _build_bass()
    nc = _cache["nc"]

    xb_cores, wq, x2, p = _host_prep(x, params, rel_idx)
    in_maps = [{"xb": xb_cores[c], "w": wq} for c in range(NCORES)]
    res = run_bass_kernel_spmd(
        nc, in_maps, list(range(NCORES)),
        trace=_trace, **({"tmpdir": _trace_out} if _trace_out else {}),
    )
    if _trace_out is not None:
        _cache["last"] = res
    y_cores = [np.asarray(res.results[c]["y"]) for c in range(NCORES)]
    return _host_post(y_cores, x2, p)
